# revision 8
# baseline (speedup 1.0000x reference)
"""Trainium2 Bass kernel: segment-reduced Euclidean loss.

loss = sum_i ||a_i - b_i||_2 / num_list[seg(i)]   over N rows, D=128.

Strategy (8 NeuronCores, data-parallel):
  - rows are split evenly across the 8 cores (segments stay whole: every
    core boundary is a multiple of the 512-row segments in the graded
    input; for general num_list the per-row weight tensor makes segment
    alignment irrelevant).
  - per core, partition p owns rows [p*q, (p+1)*q) of its shard, so each
    DMA chunk is a [128, u*128] tile whose per-partition source bytes are
    contiguous (u rows x 512B) -- large-burst, full-bandwidth DMA.
  - per chunk: VectorE subtract (in place), ScalarE Square (in place),
    VectorE grouped tensor_reduce over the innermost D=128 -> per-row
    sum-of-squares. DVE ~2 passes and ACT ~1 pass both hide under the
    ~360 GB/s HBM DMA stream.
  - tail: ScalarE Sqrt over the [128, q] sums, multiply by the per-row
    weight 1/num_list[seg(row)] (precomputed on host, DMA'd once),
    row-reduce to [128, 1], DMA out. Host sums 8x128 partials in f64.
"""

import numpy as np

N_ROWS = 1048576
D = 128
N_SEG = 2048
N_CORES = 8
ROWS_PER_CORE = N_ROWS // N_CORES  # 131072
U_DEFAULT = 32  # rows per partition per chunk


def _split_excess_waits(nc, max_waits=1):
    """walrus in this container rejects instructions carrying more than 1
    sync-wait condition ("Too many sync wait commands"). Move excess waits
    onto NoOp carrier instructions inserted just before the offender on the
    same engine -- same-engine program order makes this semantically
    identical."""
    import concourse.mybir as mybir

    for f in nc.m.functions:
        for bb in f.blocks:
            out = []
            changed = False
            for inst in bb.instructions:
                si = inst.sync_info
                waits = list(si.on_wait) if si is not None else []
                if len(waits) > max_waits:
                    keep = waits[-max_waits:]
                    extra = waits[:-max_waits]
                    k = 0
                    while extra:
                        take, extra = extra[:max_waits], extra[max_waits:]
                        nop = mybir.InstNoOp(name=f"{inst.name}-wsplit{k}")
                        nop.engine = inst.engine
                        nop.sync_info = mybir.SyncInfo(on_wait=take, on_update=[])
                        out.append(nop)
                        k += 1
                    inst.sync_info = mybir.SyncInfo(
                        on_wait=keep, on_update=list(si.on_update)
                    )
                    changed = True
                out.append(inst)
            if changed:
                bb.instructions = out


def build_nc(rows_per_core=ROWS_PER_CORE, u=U_DEFAULT, bufs=3, iters=1):
    """Build the per-core SPMD Bass program (same program on all cores).

    iters>1 repeats the streaming chunk loop (same data) for timing runs:
    HW-per-iteration = (wall[k] - wall[1]) / (k - 1) cancels launch/RPC
    overhead."""
    import concourse.bass as bass
    import concourse.mybir as mybir
    import concourse.tile as tile

    q = rows_per_core // 128  # rows per partition
    n_chunk = q // u
    assert n_chunk * u == q, (rows_per_core, u)
    f32 = mybir.dt.float32
    AF = mybir.ActivationFunctionType

    nc = bass.Bass("TRN2", target_bir_lowering=False, debug=False)
    a = nc.declare_dram_parameter("a", [rows_per_core, D], f32, isOutput=False)
    b = nc.declare_dram_parameter("b", [rows_per_core, D], f32, isOutput=False)
    w = nc.declare_dram_parameter("w", [128, q], f32, isOutput=False)
    o = nc.declare_dram_parameter("o", [128, 1], f32, isOutput=True)

    av = a.rearrange("(p q) d -> p q d", p=128)
    bv = b.rearrange("(p q) d -> p q d", p=128)

    with tile.TileContext(nc) as tc:
        with (
            tc.tile_pool(name="pa", bufs=bufs) as pa,
            tc.tile_pool(name="pb", bufs=bufs) as pb,
            tc.tile_pool(name="pers", bufs=1) as pp,
        ):
            norms = pp.tile([128, q], f32, tag="norms")
            wt = pp.tile([128, q], f32, tag="wt")
            prod = pp.tile([128, q], f32, tag="prod")
            acc = pp.tile([128, 1], f32, tag="acc")

            nc.sync.dma_start(out=wt[:], in_=w[:])

            for _ in range(iters):
                for c in range(n_chunk):
                    ta = pa.tile([128, u * D], f32)
                    tb = pb.tile([128, u * D], f32)
                    ta3 = ta[:].rearrange("p (u d) -> p u d", d=D)
                    tb3 = tb[:].rearrange("p (u d) -> p u d", d=D)
                    nc.sync.dma_start(out=ta3, in_=av[:, c * u : (c + 1) * u, :])
                    nc.sync.dma_start(out=tb3, in_=bv[:, c * u : (c + 1) * u, :])
                    nc.vector.tensor_sub(ta[:], ta[:], tb[:])
                    nc.scalar.activation(ta[:], ta[:], AF.Square)
                    nc.vector.tensor_reduce(
                        norms[:, c * u : (c + 1) * u],
                        ta3,
                        axis=mybir.AxisListType.X,
                        op=mybir.AluOpType.add,
                    )

            nc.scalar.activation(norms[:], norms[:], AF.Sqrt)
            nc.vector.tensor_mul(prod[:], norms[:], wt[:])
            nc.vector.tensor_reduce(
                acc[:], prod[:], axis=mybir.AxisListType.X, op=mybir.AluOpType.add
            )
            nc.sync.dma_start(out=o[:], in_=acc[:])

    _split_excess_waits(nc)
    return nc


def build_empty_nc():
    """Minimal program (memset + 4KB DMA out) for launch-overhead calibration."""
    import concourse.bass as bass
    import concourse.mybir as mybir
    import concourse.tile as tile

    f32 = mybir.dt.float32
    nc = bass.Bass("TRN2", target_bir_lowering=False, debug=False)
    o = nc.declare_dram_parameter("o", [128, 1], f32, isOutput=True)
    with tile.TileContext(nc) as tc:
        with tc.tile_pool(name="p", bufs=1) as pp:
            acc = pp.tile([128, 1], f32)
            nc.vector.memset(acc[:], 0.0)
            nc.sync.dma_start(out=o[:], in_=acc[:])
    _split_excess_waits(nc)
    return nc


_CACHE = {}


def _get_nc(rows_per_core, u):
    key = (rows_per_core, u)
    if key not in _CACHE:
        _CACHE[key] = build_nc(rows_per_core, u)
    return _CACHE[key]


def _seg_ids(num_list, n_rows):
    """np.repeat with jnp.repeat(total_repeat_length=n) pad/truncate semantics."""
    nl = np.asarray(num_list, dtype=np.int64)
    full = np.repeat(np.arange(nl.shape[0], dtype=np.int64), nl)
    if full.size >= n_rows:
        return full[:n_rows]
    pad_val = full[-1] if full.size else 0
    return np.concatenate([full, np.full(n_rows - full.size, pad_val, np.int64)])


def make_in_maps(clip_remap, clip_emb, num_list, rows_per_core=ROWS_PER_CORE):
    a = np.ascontiguousarray(np.asarray(clip_remap, dtype=np.float32))
    b = np.ascontiguousarray(np.asarray(clip_emb, dtype=np.float32))
    n_rows = a.shape[0]
    nl = np.asarray(num_list)
    seg = _seg_ids(nl, n_rows)
    denom = nl[seg].astype(np.float32)
    wrow = (np.float32(1.0) / denom).astype(np.float32)
    q = rows_per_core // 128
    in_maps = []
    for c in range(N_CORES):
        lo, hi = c * rows_per_core, (c + 1) * rows_per_core
        in_maps.append(
            {
                "a": a[lo:hi],
                "b": b[lo:hi],
                "w": np.ascontiguousarray(wrow[lo:hi].reshape(128, q)),
            }
        )
    return in_maps


def kernel(clip_remap, clip_emb, num_list):
    from concourse.bass_utils import run_bass_kernel_spmd

    a = np.asarray(clip_remap)
    rows_per_core = a.shape[0] // N_CORES
    nc = _get_nc(rows_per_core, U_DEFAULT)
    in_maps = make_in_maps(clip_remap, clip_emb, num_list, rows_per_core)
    res = run_bass_kernel_spmd(nc, in_maps, core_ids=list(range(N_CORES)))
    total = np.float64(0.0)
    for r in res.results:
        total += r["o"].astype(np.float64).sum()
    return np.asarray(total, dtype=np.float32)


# revision 10
# speedup vs baseline: 1.2823x; 1.2823x over previous
"""Trainium2 Bass kernel: segment-reduced Euclidean loss.

loss = sum_i ||a_i - b_i||_2 / num_list[seg(i)]   over N rows, D=128.

Strategy (8 NeuronCores, data-parallel):
  - rows are split evenly across the 8 cores (segments stay whole: every
    core boundary is a multiple of the 512-row segments in the graded
    input; for general num_list the per-row weight tensor makes segment
    alignment irrelevant).
  - per core, partition p owns rows [p*q, (p+1)*q) of its shard, so each
    DMA chunk is a [128, u*128] tile whose per-partition source bytes are
    contiguous (u rows x 512B) -- large-burst, full-bandwidth DMA.
  - per chunk: VectorE subtract (in place), ScalarE Square (in place),
    VectorE grouped tensor_reduce over the innermost D=128 -> per-row
    sum-of-squares. DVE ~2 passes and ACT ~1 pass both hide under the
    ~360 GB/s HBM DMA stream.
  - tail: ScalarE Sqrt over the [128, q] sums, multiply by the per-row
    weight 1/num_list[seg(row)] (precomputed on host, DMA'd once),
    row-reduce to [128, 1], DMA out. Host sums 8x128 partials in f64.
"""

import numpy as np

N_ROWS = 1048576
D = 128
N_SEG = 2048
N_CORES = 8
ROWS_PER_CORE = N_ROWS // N_CORES  # 131072
U_DEFAULT = 32  # rows per partition per chunk


def _split_excess_waits(nc, max_waits=1):
    """walrus in this container rejects instructions carrying more than 1
    sync-wait condition ("Too many sync wait commands"). Move excess waits
    onto NoOp carrier instructions inserted just before the offender on the
    same engine -- same-engine program order makes this semantically
    identical."""
    import concourse.mybir as mybir

    for f in nc.m.functions:
        for bb in f.blocks:
            out = []
            changed = False
            for inst in bb.instructions:
                si = inst.sync_info
                waits = list(si.on_wait) if si is not None else []
                if len(waits) > max_waits:
                    keep = waits[-max_waits:]
                    extra = waits[:-max_waits]
                    k = 0
                    while extra:
                        take, extra = extra[:max_waits], extra[max_waits:]
                        nop = mybir.InstNoOp(name=f"{inst.name}-wsplit{k}")
                        nop.engine = inst.engine
                        nop.sync_info = mybir.SyncInfo(on_wait=take, on_update=[])
                        out.append(nop)
                        k += 1
                    inst.sync_info = mybir.SyncInfo(
                        on_wait=keep, on_update=list(si.on_update)
                    )
                    changed = True
                out.append(inst)
            if changed:
                bb.instructions = out


def build_nc(rows_per_core=ROWS_PER_CORE, u=U_DEFAULT, bufs=3, iters=1):
    """Build the per-core SPMD Bass program (same program on all cores).

    iters>1 repeats the streaming chunk loop (same data) for timing runs:
    HW-per-iteration = (wall[k] - wall[1]) / (k - 1) cancels launch/RPC
    overhead."""
    import concourse.bass as bass
    import concourse.mybir as mybir
    import concourse.tile as tile

    q = rows_per_core // 128  # rows per partition
    n_chunk = q // u
    assert n_chunk * u == q, (rows_per_core, u)
    f32 = mybir.dt.float32
    AF = mybir.ActivationFunctionType

    nc = bass.Bass("TRN2", target_bir_lowering=False, debug=False)
    a = nc.declare_dram_parameter("a", [rows_per_core, D], f32, isOutput=False)
    b = nc.declare_dram_parameter("b", [rows_per_core, D], f32, isOutput=False)
    w = nc.declare_dram_parameter("w", [128, q], f32, isOutput=False)
    o = nc.declare_dram_parameter("o", [128, 1], f32, isOutput=True)

    av = a.rearrange("(p q) d -> p q d", p=128)
    bv = b.rearrange("(p q) d -> p q d", p=128)

    with tile.TileContext(nc) as tc:
        with (
            tc.tile_pool(name="pa", bufs=bufs) as pa,
            tc.tile_pool(name="pb", bufs=bufs) as pb,
            tc.tile_pool(name="pers", bufs=1) as pp,
        ):
            norms = pp.tile([128, q], f32, tag="norms")
            wt = pp.tile([128, q], f32, tag="wt")
            prod = pp.tile([128, q], f32, tag="prod")
            acc = pp.tile([128, 1], f32, tag="acc")

            nc.sync.dma_start(out=wt[:], in_=w[:])

            for _ in range(iters):
                # Software-pipelined emission: the grouped reduce for chunk
                # c-1 is emitted after chunk c's subtract, so the DVE stream
                # never head-of-line blocks on ACT's Square (DVE order is
                # program order; red_c directly after sub_c would stall DVE
                # for the full ACT pass each chunk).
                pending = None  # (ta3, norms_slice) awaiting reduce
                for c in range(n_chunk):
                    ta = pa.tile([128, u * D], f32)
                    tb = pb.tile([128, u * D], f32)
                    ta3 = ta[:].rearrange("p (u d) -> p u d", d=D)
                    tb3 = tb[:].rearrange("p (u d) -> p u d", d=D)
                    nc.sync.dma_start(out=ta3, in_=av[:, c * u : (c + 1) * u, :])
                    nc.sync.dma_start(out=tb3, in_=bv[:, c * u : (c + 1) * u, :])
                    nc.vector.tensor_sub(ta[:], ta[:], tb[:])
                    if pending is not None:
                        nc.vector.tensor_reduce(
                            pending[1],
                            pending[0],
                            axis=mybir.AxisListType.X,
                            op=mybir.AluOpType.add,
                        )
                    nc.scalar.activation(ta[:], ta[:], AF.Square)
                    pending = (ta3, norms[:, c * u : (c + 1) * u])
                nc.vector.tensor_reduce(
                    pending[1],
                    pending[0],
                    axis=mybir.AxisListType.X,
                    op=mybir.AluOpType.add,
                )

            nc.scalar.activation(norms[:], norms[:], AF.Sqrt)
            nc.vector.tensor_mul(prod[:], norms[:], wt[:])
            nc.vector.tensor_reduce(
                acc[:], prod[:], axis=mybir.AxisListType.X, op=mybir.AluOpType.add
            )
            nc.sync.dma_start(out=o[:], in_=acc[:])

    _split_excess_waits(nc)
    return nc


def build_empty_nc():
    """Minimal program (memset + 4KB DMA out) for launch-overhead calibration."""
    import concourse.bass as bass
    import concourse.mybir as mybir
    import concourse.tile as tile

    f32 = mybir.dt.float32
    nc = bass.Bass("TRN2", target_bir_lowering=False, debug=False)
    o = nc.declare_dram_parameter("o", [128, 1], f32, isOutput=True)
    with tile.TileContext(nc) as tc:
        with tc.tile_pool(name="p", bufs=1) as pp:
            acc = pp.tile([128, 1], f32)
            nc.vector.memset(acc[:], 0.0)
            nc.sync.dma_start(out=o[:], in_=acc[:])
    _split_excess_waits(nc)
    return nc


_CACHE = {}


def _get_nc(rows_per_core, u):
    key = (rows_per_core, u)
    if key not in _CACHE:
        _CACHE[key] = build_nc(rows_per_core, u)
    return _CACHE[key]


def _seg_ids(num_list, n_rows):
    """np.repeat with jnp.repeat(total_repeat_length=n) pad/truncate semantics."""
    nl = np.asarray(num_list, dtype=np.int64)
    full = np.repeat(np.arange(nl.shape[0], dtype=np.int64), nl)
    if full.size >= n_rows:
        return full[:n_rows]
    pad_val = full[-1] if full.size else 0
    return np.concatenate([full, np.full(n_rows - full.size, pad_val, np.int64)])


def make_in_maps(clip_remap, clip_emb, num_list, rows_per_core=ROWS_PER_CORE):
    a = np.ascontiguousarray(np.asarray(clip_remap, dtype=np.float32))
    b = np.ascontiguousarray(np.asarray(clip_emb, dtype=np.float32))
    n_rows = a.shape[0]
    nl = np.asarray(num_list)
    seg = _seg_ids(nl, n_rows)
    denom = nl[seg].astype(np.float32)
    wrow = (np.float32(1.0) / denom).astype(np.float32)
    q = rows_per_core // 128
    in_maps = []
    for c in range(N_CORES):
        lo, hi = c * rows_per_core, (c + 1) * rows_per_core
        in_maps.append(
            {
                "a": a[lo:hi],
                "b": b[lo:hi],
                "w": np.ascontiguousarray(wrow[lo:hi].reshape(128, q)),
            }
        )
    return in_maps


_RUNNER_CACHE = {}


def _get_runner(rows_per_core, u=U_DEFAULT):
    """Compile once per process; reuse the jitted SPMD executable across
    kernel() calls (run_bass_kernel_spmd re-traces and re-compiles on every
    invocation because it builds a fresh closure)."""
    key = (rows_per_core, u)
    if key in _RUNNER_CACHE:
        return _RUNNER_CACHE[key]

    import jax
    from jax.experimental.shard_map import shard_map
    from jax.sharding import Mesh, NamedSharding, PartitionSpec

    import concourse.bass2jax as b2j
    import concourse.mybir as mybir

    b2j.install_neuronx_cc_hook()
    nc = _get_nc(rows_per_core, u)

    in_names, out_names, out_avals, zero_outs = [], [], [], []
    pname = nc.partition_id_tensor.name if nc.partition_id_tensor else None
    for alloc in nc.m.functions[0].allocations:
        if not isinstance(alloc, mybir.MemoryLocationSet):
            continue
        name = alloc.memorylocations[0].name
        if alloc.kind == "ExternalInput":
            if name != pname:
                in_names.append(name)
        elif alloc.kind == "ExternalOutput":
            out_names.append(name)
            shape = tuple(alloc.tensor_shape)
            dtype = mybir.dt.np(alloc.dtype)
            out_avals.append(jax.core.ShapedArray(shape, dtype))
            zero_outs.append(np.zeros(shape, dtype))
    n_params = len(in_names)
    all_in = list(in_names) + list(out_names)
    if pname is not None:
        all_in.append(pname)

    def _body(*args):
        operands = list(args)
        if pname is not None:
            operands.append(b2j.partition_id_tensor())
        return tuple(
            b2j._bass_exec_p.bind(
                *operands,
                out_avals=tuple(out_avals),
                in_names=tuple(all_in),
                out_names=tuple(out_names),
                lowering_input_output_aliases=(),
                sim_require_finite=True,
                sim_require_nnan=True,
                nc=nc,
            )
        )

    devices = jax.devices()[:N_CORES]
    mesh = Mesh(np.asarray(devices), ("core",))
    n_outs = len(out_avals)
    fn = jax.jit(
        shard_map(
            _body,
            mesh=mesh,
            in_specs=(PartitionSpec("core"),) * (n_params + n_outs),
            out_specs=(PartitionSpec("core"),) * n_outs,
            check_rep=False,
        ),
        keep_unused=True,
    )
    sh = NamedSharding(mesh, PartitionSpec("core"))

    def run(in_maps):
        dev_in = [
            jax.device_put(
                np.concatenate([np.asarray(m[nm]) for m in in_maps], axis=0), sh
            )
            for nm in in_names
        ]
        dev_zero = [
            jax.device_put(np.concatenate([z] * N_CORES, axis=0), sh)
            for z in zero_outs
        ]
        outs = fn(*dev_in, *dev_zero)
        jax.block_until_ready(outs)
        # split concatenated outputs back per core
        results = []
        for c in range(N_CORES):
            r = {}
            for i, nm in enumerate(out_names):
                arr = np.asarray(outs[i])
                per = arr.shape[0] // N_CORES
                r[nm] = arr[c * per : (c + 1) * per]
            results.append(r)
        return results

    _RUNNER_CACHE[key] = run
    return run


def kernel(clip_remap, clip_emb, num_list):
    a = np.asarray(clip_remap)
    rows_per_core = a.shape[0] // N_CORES
    in_maps = make_in_maps(clip_remap, clip_emb, num_list, rows_per_core)
    run = _get_runner(rows_per_core, U_DEFAULT)
    try:
        results = run(in_maps)
    except Exception:
        # one retry: transient NRT/axon failures have been observed
        results = run(in_maps)
    total = np.float64(0.0)
    for r in results:
        total += r["o"].astype(np.float64).sum()
    return np.asarray(total, dtype=np.float32)


# revision 12
# speedup vs baseline: 1.3042x; 1.0171x over previous
"""Trainium2 Bass kernel: segment-reduced Euclidean loss.

loss = sum_i ||a_i - b_i||_2 / num_list[seg(i)]   over N rows, D=128.

Strategy (8 NeuronCores, data-parallel):
  - rows are split evenly across the 8 cores (segments stay whole: every
    core boundary is a multiple of the 512-row segments in the graded
    input; for general num_list the per-row weight tensor makes segment
    alignment irrelevant).
  - per core, partition p owns rows [p*q, (p+1)*q) of its shard, so each
    DMA chunk is a [128, u*128] tile whose per-partition source bytes are
    contiguous (u rows x 512B) -- large-burst, full-bandwidth DMA.
  - per chunk: VectorE subtract (in place), ScalarE Square (in place),
    VectorE grouped tensor_reduce over the innermost D=128 -> per-row
    sum-of-squares. DVE ~2 passes and ACT ~1 pass both hide under the
    ~360 GB/s HBM DMA stream.
  - tail: ScalarE Sqrt over the [128, q] sums, multiply by the per-row
    weight 1/num_list[seg(row)] (precomputed on host, DMA'd once),
    row-reduce to [128, 1], DMA out. Host sums 8x128 partials in f64.
"""

import numpy as np

N_ROWS = 1048576
D = 128
N_SEG = 2048
N_CORES = 8
ROWS_PER_CORE = N_ROWS // N_CORES  # 131072
U_DEFAULT = 32  # rows per partition per chunk


def _split_excess_waits(nc, max_waits=1):
    """walrus in this container rejects instructions carrying more than 1
    sync-wait condition ("Too many sync wait commands"). Move excess waits
    onto NoOp carrier instructions inserted just before the offender on the
    same engine -- same-engine program order makes this semantically
    identical."""
    import concourse.mybir as mybir

    for f in nc.m.functions:
        for bb in f.blocks:
            out = []
            changed = False
            for inst in bb.instructions:
                si = inst.sync_info
                waits = list(si.on_wait) if si is not None else []
                if len(waits) > max_waits:
                    keep = waits[-max_waits:]
                    extra = waits[:-max_waits]
                    k = 0
                    while extra:
                        take, extra = extra[:max_waits], extra[max_waits:]
                        nop = mybir.InstNoOp(name=f"{inst.name}-wsplit{k}")
                        nop.engine = inst.engine
                        nop.sync_info = mybir.SyncInfo(on_wait=take, on_update=[])
                        out.append(nop)
                        k += 1
                    inst.sync_info = mybir.SyncInfo(
                        on_wait=keep, on_update=list(si.on_update)
                    )
                    changed = True
                out.append(inst)
            if changed:
                bb.instructions = out


def build_nc(rows_per_core=ROWS_PER_CORE, u=U_DEFAULT, bufs=3, iters=1):
    """Build the per-core SPMD Bass program (same program on all cores).

    iters>1 repeats the streaming chunk loop (same data) for timing runs:
    HW-per-iteration = (wall[k] - wall[1]) / (k - 1) cancels launch/RPC
    overhead."""
    import concourse.bass as bass
    import concourse.mybir as mybir
    import concourse.tile as tile

    q = rows_per_core // 128  # rows per partition
    n_chunk = q // u
    assert n_chunk * u == q, (rows_per_core, u)
    f32 = mybir.dt.float32
    AF = mybir.ActivationFunctionType

    nc = bass.Bass("TRN2", target_bir_lowering=False, debug=False)
    a = nc.declare_dram_parameter("a", [rows_per_core, D], f32, isOutput=False)
    b = nc.declare_dram_parameter("b", [rows_per_core, D], f32, isOutput=False)
    w = nc.declare_dram_parameter("w", [128, q], f32, isOutput=False)
    o = nc.declare_dram_parameter("o", [128, 1], f32, isOutput=True)

    av = a.rearrange("(p q) d -> p q d", p=128)
    bv = b.rearrange("(p q) d -> p q d", p=128)

    with tile.TileContext(nc) as tc:
        with (
            tc.tile_pool(name="pa", bufs=bufs) as pa,
            tc.tile_pool(name="pb", bufs=bufs) as pb,
            tc.tile_pool(name="pers", bufs=1) as pp,
        ):
            norms = pp.tile([128, q], f32, tag="norms")
            wt = pp.tile([128, q], f32, tag="wt")
            prod = pp.tile([128, q], f32, tag="prod")
            acc = pp.tile([128, 1], f32, tag="acc")

            nc.sync.dma_start(out=wt[:], in_=w[:])

            for _ in range(iters):
                # Software-pipelined emission: the grouped reduce for chunk
                # c-1 is emitted after chunk c's subtract, so the DVE stream
                # never head-of-line blocks on ACT's Square (DVE order is
                # program order; red_c directly after sub_c would stall DVE
                # for the full ACT pass each chunk).
                pending = None  # (ta3, norms_slice) awaiting reduce
                for c in range(n_chunk):
                    ta = pa.tile([128, u * D], f32)
                    tb = pb.tile([128, u * D], f32)
                    ta3 = ta[:].rearrange("p (u d) -> p u d", d=D)
                    tb3 = tb[:].rearrange("p (u d) -> p u d", d=D)
                    nc.sync.dma_start(out=ta3, in_=av[:, c * u : (c + 1) * u, :])
                    nc.sync.dma_start(out=tb3, in_=bv[:, c * u : (c + 1) * u, :])
                    nc.vector.tensor_sub(ta[:], ta[:], tb[:])
                    if pending is not None:
                        nc.vector.tensor_reduce(
                            pending[1],
                            pending[0],
                            axis=mybir.AxisListType.X,
                            op=mybir.AluOpType.add,
                        )
                    nc.scalar.activation(ta[:], ta[:], AF.Square)
                    pending = (ta3, norms[:, c * u : (c + 1) * u])
                nc.vector.tensor_reduce(
                    pending[1],
                    pending[0],
                    axis=mybir.AxisListType.X,
                    op=mybir.AluOpType.add,
                )

            nc.scalar.activation(norms[:], norms[:], AF.Sqrt)
            nc.vector.tensor_mul(prod[:], norms[:], wt[:])
            nc.vector.tensor_reduce(
                acc[:], prod[:], axis=mybir.AxisListType.X, op=mybir.AluOpType.add
            )
            nc.sync.dma_start(out=o[:], in_=acc[:])

    _split_excess_waits(nc)
    return nc


def build_empty_nc():
    """Minimal program (memset + 4KB DMA out) for launch-overhead calibration."""
    import concourse.bass as bass
    import concourse.mybir as mybir
    import concourse.tile as tile

    f32 = mybir.dt.float32
    nc = bass.Bass("TRN2", target_bir_lowering=False, debug=False)
    o = nc.declare_dram_parameter("o", [128, 1], f32, isOutput=True)
    with tile.TileContext(nc) as tc:
        with tc.tile_pool(name="p", bufs=1) as pp:
            acc = pp.tile([128, 1], f32)
            nc.vector.memset(acc[:], 0.0)
            nc.sync.dma_start(out=o[:], in_=acc[:])
    _split_excess_waits(nc)
    return nc


_CACHE = {}


def _get_nc(rows_per_core, u):
    key = (rows_per_core, u)
    if key not in _CACHE:
        _CACHE[key] = build_nc(rows_per_core, u)
    return _CACHE[key]


def _seg_ids(num_list, n_rows):
    """np.repeat with jnp.repeat(total_repeat_length=n) pad/truncate semantics."""
    nl = np.asarray(num_list, dtype=np.int64)
    full = np.repeat(np.arange(nl.shape[0], dtype=np.int64), nl)
    if full.size >= n_rows:
        return full[:n_rows]
    pad_val = full[-1] if full.size else 0
    return np.concatenate([full, np.full(n_rows - full.size, pad_val, np.int64)])


def make_in_maps(clip_remap, clip_emb, num_list, rows_per_core=ROWS_PER_CORE):
    a = np.ascontiguousarray(np.asarray(clip_remap, dtype=np.float32))
    b = np.ascontiguousarray(np.asarray(clip_emb, dtype=np.float32))
    n_rows = a.shape[0]
    nl = np.asarray(num_list)
    seg = _seg_ids(nl, n_rows)
    denom = nl[seg].astype(np.float32)
    wrow = (np.float32(1.0) / denom).astype(np.float32)
    q = rows_per_core // 128
    in_maps = []
    for c in range(N_CORES):
        lo, hi = c * rows_per_core, (c + 1) * rows_per_core
        in_maps.append(
            {
                "a": a[lo:hi],
                "b": b[lo:hi],
                "w": np.ascontiguousarray(wrow[lo:hi].reshape(128, q)),
            }
        )
    return in_maps


_RUNNER_CACHE = {}


def _get_runner(rows_per_core, u=U_DEFAULT):
    """Compile once per process; reuse the jitted SPMD executable across
    kernel() calls (run_bass_kernel_spmd re-traces and re-compiles on every
    invocation because it builds a fresh closure)."""
    key = (rows_per_core, u)
    if key in _RUNNER_CACHE:
        return _RUNNER_CACHE[key]

    import jax
    from jax.experimental.shard_map import shard_map
    from jax.sharding import Mesh, NamedSharding, PartitionSpec

    import concourse.bass2jax as b2j
    import concourse.mybir as mybir

    b2j.install_neuronx_cc_hook()
    nc = _get_nc(rows_per_core, u)

    in_names, out_names, out_avals, zero_outs = [], [], [], []
    pname = nc.partition_id_tensor.name if nc.partition_id_tensor else None
    for alloc in nc.m.functions[0].allocations:
        if not isinstance(alloc, mybir.MemoryLocationSet):
            continue
        name = alloc.memorylocations[0].name
        if alloc.kind == "ExternalInput":
            if name != pname:
                in_names.append(name)
        elif alloc.kind == "ExternalOutput":
            out_names.append(name)
            shape = tuple(alloc.tensor_shape)
            dtype = mybir.dt.np(alloc.dtype)
            out_avals.append(jax.core.ShapedArray(shape, dtype))
            zero_outs.append(np.zeros(shape, dtype))
    n_params = len(in_names)
    all_in = list(in_names) + list(out_names)
    if pname is not None:
        all_in.append(pname)

    def _body(*args):
        operands = list(args)
        if pname is not None:
            operands.append(b2j.partition_id_tensor())
        return tuple(
            b2j._bass_exec_p.bind(
                *operands,
                out_avals=tuple(out_avals),
                in_names=tuple(all_in),
                out_names=tuple(out_names),
                lowering_input_output_aliases=(),
                sim_require_finite=True,
                sim_require_nnan=True,
                nc=nc,
            )
        )

    devices = jax.devices()[:N_CORES]
    mesh = Mesh(np.asarray(devices), ("core",))
    n_outs = len(out_avals)
    fn = jax.jit(
        shard_map(
            _body,
            mesh=mesh,
            in_specs=(PartitionSpec("core"),) * (n_params + n_outs),
            out_specs=(PartitionSpec("core"),) * n_outs,
            check_rep=False,
        ),
        keep_unused=True,
    )
    sh = NamedSharding(mesh, PartitionSpec("core"))

    def run(in_maps):
        dev_in = [
            jax.device_put(
                np.concatenate([np.asarray(m[nm]) for m in in_maps], axis=0), sh
            )
            for nm in in_names
        ]
        dev_zero = [
            jax.device_put(np.concatenate([z] * N_CORES, axis=0), sh)
            for z in zero_outs
        ]
        outs = fn(*dev_in, *dev_zero)
        jax.block_until_ready(outs)
        # split concatenated outputs back per core
        results = []
        for c in range(N_CORES):
            r = {}
            for i, nm in enumerate(out_names):
                arr = np.asarray(outs[i])
                per = arr.shape[0] // N_CORES
                r[nm] = arr[c * per : (c + 1) * per]
            results.append(r)
        return results

    _RUNNER_CACHE[key] = run
    return run


def kernel(clip_remap, clip_emb, num_list):
    a = np.asarray(clip_remap)
    rows_per_core = a.shape[0] // N_CORES
    in_maps = make_in_maps(clip_remap, clip_emb, num_list, rows_per_core)
    results = None
    last_err = None
    for attempt in range(4):
        try:
            if attempt < 3:
                run = _get_runner(rows_per_core, U_DEFAULT)
                results = run(in_maps)
            else:
                # last resort: the stock SPMD runner (fresh trace/compile)
                from concourse.bass_utils import run_bass_kernel_spmd

                res = run_bass_kernel_spmd(
                    _get_nc(rows_per_core, U_DEFAULT),
                    in_maps,
                    core_ids=list(range(N_CORES)),
                )
                results = res.results
            break
        except Exception as e:  # transient NRT/axon failures observed
            last_err = e
            import time as _time

            _time.sleep(2.0 * (attempt + 1))
            if attempt >= 1:
                # rebuild the jitted executable from scratch
                _RUNNER_CACHE.pop((rows_per_core, U_DEFAULT), None)
    if results is None:
        raise last_err
    total = np.float64(0.0)
    for r in results:
        total += r["o"].astype(np.float64).sum()
    return np.asarray(total, dtype=np.float32)
